# revision 1
# baseline (speedup 1.0000x reference)
"""Trainium2 Bass kernel for nn_ModelConTT_46016279609475 (TT interpolation).

y[b] = v0[b]^T V1[b] V2[b] v3[b], where v_i are linearly-interpolated slices
of tiny TT cores at per-point grid coordinates derived from x[b, :].

Strategy (per NeuronCore, data-parallel over B):
  * Precompute joint tables on device with PE matmuls:
      G[n0, n1, k] = sum_c core0[n0, c] * core1[c, n1, k]      (u-side)
      H[n3, n2, k] = sum_c core3[c, n3] * core2[k, n2, c]      (v-side)
    stored f32 in DRAM as 4-corner-packed 256B entries [dn0, dn1, k]:
      G4[(n0*128+n1), (dn0, dn1, k)] = G[n0+dn0, n1+dn1, k]
    so one dma_gather element fetches everything needed for the bilinear
    interpolation of u[b] (and same for v[b]).
  * Per point: idx = lo0*128 + lo1 (int16), one 256B dma_gather per table,
    DVE does the 4-corner weighted sum and the final k-dot:
      y[b] = sum_k (sum_c wG_c gG[c,k]) * (sum_c wH_c gH[c,k])

Batch mapping per core: shard b of size 32768; on-chip layout is
"p-minor": element i lives at partition i%128, free col i//128, matching
dma_gather's output layout dst[i%128, i//128]. Index lists are mod-16
wrapped as dma_gather requires (idx for i at [i%16, i//16]) and replicated
across all 8 Q7 core groups (each SWDGE core pair reads its own 16 rows).

Exact-floor trick (f32-safe): t = (xc + 2^23) - 2^23 rounds to nearest;
g = (t > xc); floor = t - g; frac = xc - floor computed via the exact
(t1 - 2^23) path to avoid re-rounding.
"""

import numpy as np
import ml_dtypes

import concourse.bass as bass
import concourse.bacc as bacc
import concourse.mybir as mybir
import concourse.tile as tile
from concourse import library_config
from concourse.bass_utils import run_bass_kernel_spmd

F32 = mybir.dt.float32
BF16 = mybir.dt.bfloat16
I16 = mybir.dt.int16
OP = mybir.AluOpType
AF = mybir.ActivationFunctionType

NCORES = 8
B = 262144
BS = B // NCORES          # 32768 points per core
P = 128                   # partitions
J = BS // P               # 256 free cols per partition
NCH = 8                   # pipeline chunks
JC = J // NCH             # 32 cols per chunk
NIDX = P * JC             # 4096 idxs per gather
LC = NIDX // 16           # 256 idx-list cols per chunk
N = 128                   # mode size
R = 16                    # TT rank
TE = N * N                # table entries
ES = 64                   # f32 elems per entry: 4 corners x 16 k = 256B
MAGIC = float(2 ** 23)
SCALE = (N - 1) / 2.0     # 63.5
M16 = BS // 16            # 2048 idx-list cols total

_CACHED = None
DEBUG_TILES = {}


def _build_nc(stage="full"):
    nc = bacc.Bacc("TRN2")

    x_pm = nc.dram_tensor("x_pm", [P, J, 4], F32, kind="ExternalInput")
    xq = nc.dram_tensor("xq", [64, M16 // 2, 2], F32, kind="ExternalInput")
    c0t = nc.dram_tensor("c0t", [16, 129], F32, kind="ExternalInput")
    c1f = nc.dram_tensor("c1f", [16, 2096], F32, kind="ExternalInput")
    c3f = nc.dram_tensor("c3f", [16, 129], F32, kind="ExternalInput")
    c2t = nc.dram_tensor("c2t", [16, 2096], F32, kind="ExternalInput")
    y_pm = nc.dram_tensor("y_pm", [P, J], F32, kind="ExternalOutput")

    with tile.TileContext(nc) as tc:
        with (
            tc.tile_pool(name="per", bufs=1) as pe,
            tc.tile_pool(name="ps", bufs=4, space="PSUM") as pp,
            tc.tile_pool(name="dr", bufs=1, space="DRAM") as dp,
        ):
            nc.gpsimd.load_library(library_config.mlp)

            # persistent tiles (lists fully memset once: the gather idx AP
            # spans all 128 partitions but HW only reads rows 0-31, its
            # queue's core pair; the sim reads rows 0-15)
            LG = pe.tile([P, M16], I16)
            LH = pe.tile([P, M16], I16)
            nc.vector.memset(LG[:], 0)
            nc.scalar.memzero(LH[:])
            WG = pe.tile([P, 4, J], F32)
            WH = pe.tile([P, 4, J], F32)
            G4 = pe.tile([P, N, ES], F32)
            H4 = pe.tile([P, N, ES], F32)
            ysb = pe.tile([P, J], F32)
            g4d = dp.tile([TE, ES], F32)
            h4d = dp.tile([TE, ES], F32)

            with tc.tile_pool(name="pre", bufs=1) as wp:
                # ------------- constant loads -------------
                c0t_s = wp.tile([16, 129], F32)
                nc.sync.dma_start(c0t_s[:], c0t[:])
                c1f_s = wp.tile([16, 2096], F32)
                nc.sync.dma_start(c1f_s[:], c1f[:])
                c3f_s = wp.tile([16, 129], F32)
                nc.sync.dma_start(c3f_s[:], c3f[:])
                c2t_s = wp.tile([16, 2096], F32)
                nc.sync.dma_start(c2t_s[:], c2t[:])
                x_s = wp.tile([P, J * 4], F32)
                nc.sync.dma_start(x_s[:], x_pm[:].rearrange("p a b -> p (a b)"))
                xq_s = wp.tile([112, M16], F32)
                nc.vector.memset(xq_s[:], 0.0)
                xqv = xq[:].rearrange("p a b -> p (a b)")
                nc.sync.dma_start(xq_s[0:16, :], xqv[0:16, :])
                nc.sync.dma_start(xq_s[32:48, :], xqv[16:32, :])
                nc.sync.dma_start(xq_s[64:80, :], xqv[32:48, :])
                nc.sync.dma_start(xq_s[96:112, :], xqv[48:64, :])

                # ------------- table build ----------------
                # chunk-outer so each n1-quarter's DRAM write starts as soon
                # as its four corner copies land (overlaps write with build).
                for tbl, tdr, lhs, rhs in (
                    (G4, g4d, c0t_s, c1f_s),
                    (H4, h4d, c3f_s, c2t_s),
                ):
                    tblv = tbl[:].rearrange("p n (c k) -> p n c k", k=R)
                    tdrv = tdr[:].rearrange("(p a) b -> p (a b)", p=P)
                    tsbv = tbl[:].rearrange("p a b -> p (a b)")
                    for ch in range(4):
                        for ci, (dhi, dlo) in enumerate(
                            ((0, 0), (0, 1), (1, 0), (1, 1))
                        ):
                            ps = pp.tile([P, 512], F32, tag="mmps")
                            nc.tensor.matmul(
                                ps[:],
                                lhs[:, dhi : dhi + 128],
                                rhs[
                                    :,
                                    16 * dlo + 512 * ch : 16 * dlo + 512 * ch + 512,
                                ],
                                start=True,
                                stop=True,
                            )
                            dst = tblv[:, 32 * ch : 32 * ch + 32, ci, :]
                            src = ps[:].rearrange("p (a b) -> p a b", b=R)
                            if ci % 2 == 0:
                                nc.vector.tensor_copy(dst, src)
                            else:
                                nc.scalar.copy(dst, src)
                        nc.sync.dma_start(
                            tdrv[:, 2048 * ch : 2048 * ch + 2048],
                            tsbv[:, 2048 * ch : 2048 * ch + 2048],
                        )

                # ------------- index lists ----------------
                # four 16-row bands (at partition bases 0/32/64/96 -- the
                # only legal compute starts): G cols 0-1023 / G cols
                # 1024-2047 / H cols 0-1023 / H cols 1024-2047. Halves the
                # per-op free size vs a single band.
                nc.vector.tensor_scalar(
                    xq_s[:], xq_s[:], SCALE, SCALE, OP.mult, OP.add
                )
                t1q = wp.tile([112, M16], F32)
                nc.scalar.activation(t1q[:], xq_s[:], AF.Copy, bias=MAGIC, scale=1.0)
                gq = wp.tile([112, M16], F32)
                nc.vector.scalar_tensor_tensor(
                    gq[:], t1q[:], -MAGIC, xq_s[:], OP.add, OP.is_gt
                )
                # lo = (t1 - MAGIC) - g  (exact floor), in place over t1q
                nc.vector.scalar_tensor_tensor(
                    t1q[:], t1q[:], -MAGIC, gq[:], OP.add, OP.subtract
                )
                # idx = lo_hi*128 + lo_lo, int16 cast fused into the op's
                # output dtype; written straight into the list tiles.
                lo_hi = t1q[:].rearrange("p (m two) -> p m two", two=2)
                H16 = M16 // 2
                for band, dst in (
                    (0, LG[0:16, 0:H16]),
                    (32, LG[32:48, H16:M16]),
                    (64, LH[64:80, 0:H16]),
                    (96, LH[96:112, H16:M16]),
                ):
                    nc.vector.scalar_tensor_tensor(
                        dst,
                        lo_hi[band : band + 16, :, 0],
                        128.0,
                        lo_hi[band : band + 16, :, 1],
                        OP.mult,
                        OP.add,
                    )
                nc.sync.dma_start(LG[0:16, H16:M16], LG[32:48, H16:M16])
                nc.sync.dma_start(LG[16:32, :], LG[0:16, :])
                nc.sync.dma_start(LH[0:16, 0:H16], LH[64:80, 0:H16])
                nc.sync.dma_start(LH[0:16, H16:M16], LH[96:112, H16:M16])
                nc.sync.dma_start(LH[16:32, :], LH[0:16, :])

                # ------------- interp weights -------------
                # x_s is [128, (256 j, 4 d)]; w = frac(xc), a = 1 - w.
                nc.vector.tensor_scalar(
                    x_s[:], x_s[:], SCALE, SCALE, OP.mult, OP.add
                )
                t1 = wp.tile([P, J * 4], F32)
                nc.scalar.activation(t1[:], x_s[:], AF.Copy, bias=MAGIC, scale=1.0)
                gw = wp.tile([P, J * 4], F32)
                nc.vector.scalar_tensor_tensor(
                    gw[:], t1[:], -MAGIC, x_s[:], OP.add, OP.is_gt
                )
                # s1 = (t1 - MAGIC) - xc = t - xc   (exact: t1 - MAGIC is exact)
                s1 = wp.tile([P, J * 4], F32)
                nc.vector.scalar_tensor_tensor(
                    s1[:], t1[:], -MAGIC, x_s[:], OP.add, OP.subtract
                )
                # w = g - (t - xc) = xc - floor(xc), in place over s1
                nc.vector.tensor_tensor(s1[:], gw[:], s1[:], OP.subtract)
                aw = wp.tile([P, J * 4], F32, tag="t1")
                nc.vector.tensor_scalar(aw[:], s1[:], -1.0, 1.0, OP.mult, OP.add)

                wv = s1[:].rearrange("p (j d) -> p j d", d=4)
                av = aw[:].rearrange("p (j d) -> p j d", d=4)
                # G corners (dn0, dn1): (a0,a1),(a0,w1),(w0,a1),(w0,w1)
                nc.vector.tensor_tensor(WG[:, 0, :], av[:, :, 0], av[:, :, 1], OP.mult)
                nc.vector.tensor_tensor(WG[:, 1, :], av[:, :, 0], wv[:, :, 1], OP.mult)
                nc.vector.tensor_tensor(WG[:, 2, :], wv[:, :, 0], av[:, :, 1], OP.mult)
                nc.vector.tensor_tensor(WG[:, 3, :], wv[:, :, 0], wv[:, :, 1], OP.mult)
                # H corners (dn3, dn2): (a3,a2),(a3,w2),(w3,a2),(w3,w2)
                nc.vector.tensor_tensor(WH[:, 0, :], av[:, :, 3], av[:, :, 2], OP.mult)
                nc.vector.tensor_tensor(WH[:, 1, :], av[:, :, 3], wv[:, :, 2], OP.mult)
                nc.vector.tensor_tensor(WH[:, 2, :], wv[:, :, 3], av[:, :, 2], OP.mult)
                nc.vector.tensor_tensor(WH[:, 3, :], wv[:, :, 3], wv[:, :, 2], OP.mult)

            # ------------- gather + combine ---------------
            if stage != "full":
                nc.vector.memset(ysb[:], 0.0)
            nch = {"tables": 0, "gather1": 1}.get(stage, NCH)
            with (
                tc.tile_pool(name="gbuf", bufs=3) as gb,
                tc.tile_pool(name="cbuf", bufs=2) as cb,
            ):
                for ch in range(nch):
                    gGt = gb.tile([P, JC, ES], F32, tag="gG")
                    nc.gpsimd.dma_gather(
                        gGt[:],
                        g4d[:],
                        LG[:, LC * ch : LC * ch + LC],
                        NIDX,
                        NIDX,
                        ES,
                        queue_num=0,
                        single_packet=False,
                    )
                    gHt = gb.tile([P, JC, ES], F32, tag="gH")
                    nc.gpsimd.dma_gather(
                        gHt[:],
                        h4d[:],
                        LH[:, LC * ch : LC * ch + LC],
                        NIDX,
                        NIDX,
                        ES,
                        queue_num=0,
                        single_packet=False,
                    )
                    gG = gGt[:]
                    gH = gHt[:]

                    uv = []
                    for ti, (g, W) in enumerate(((gG, WG), (gH, WH))):
                        # m[c, j, k] = gathered corner value * corner weight
                        # (weight broadcast over k via stride-0 AP; no
                        # materialization)
                        m = cb.tile([P, 4, JC, R], F32, tag=f"m{ti}")
                        gv = g.rearrange("p j (c k) -> p c j k", c=4)
                        wbc = (
                            W[:, :, JC * ch : JC * ch + JC]
                            .unsqueeze(3)
                            .broadcast_to([P, 4, JC, R])
                        )
                        nc.vector.tensor_tensor(m[:], gv, wbc, OP.mult)
                        t2 = cb.tile([P, 2, JC, R], F32, tag=f"t{ti}")
                        nc.vector.tensor_tensor(
                            t2[:], m[:, 0:2], m[:, 2:4], OP.add
                        )
                        u = cb.tile([P, JC, R], F32, tag=f"u{ti}")
                        nc.vector.tensor_tensor(u[:], t2[:, 0], t2[:, 1], OP.add)
                        uv.append(u)

                    pr = cb.tile([P, JC, R], F32, tag="pr")
                    nc.vector.tensor_tensor(pr[:], uv[0][:], uv[1][:], OP.mult)
                    nc.vector.tensor_reduce(
                        ysb[:, JC * ch : JC * ch + JC],
                        pr[:],
                        mybir.AxisListType.X,
                        OP.add,
                    )

            nc.sync.dma_start(y_pm[:], ysb[:])
            DEBUG_TILES.update(LG=LG, LH=LH, WG=WG, WH=WH, G4=G4, H4=H4,
                               ysb=ysb, g4d=g4d, h4d=h4d)

    nc.finalize()
    return nc


def _prep_inputs(x, core0, core1, core2, core3):
    """Host-side input marshalling: shard x over cores, lay out tensors in
    the on-chip layouts the kernel expects, pad core matrices for the
    shifted-corner matmuls (cast to bf16 on host)."""
    xs = np.ascontiguousarray(np.asarray(x, dtype=np.float32).reshape(NCORES, BS, 4))

    core0 = np.asarray(core0, dtype=np.float32)
    core1 = np.asarray(core1, dtype=np.float32)
    core2 = np.asarray(core2, dtype=np.float32)
    core3 = np.asarray(core3, dtype=np.float32)

    c0 = core0[0]                        # [128, 16]
    c0t = np.ascontiguousarray(np.concatenate([c0.T, c0.T[:, -1:]], axis=1))
    c1 = core1.reshape(16, 2048)
    c1f = np.ascontiguousarray(
        np.concatenate([c1, np.tile(c1[:, -16:], (1, 3))], axis=1)
    )
    c2 = np.ascontiguousarray(core2.transpose(2, 1, 0)).reshape(16, 2048)
    c2t = np.ascontiguousarray(
        np.concatenate([c2, np.tile(c2[:, -16:], (1, 3))], axis=1)
    )
    c3 = core3[:, :, 0]                  # [16, 128]
    c3f = np.ascontiguousarray(np.concatenate([c3, c3[:, -1:]], axis=1))

    in_maps = []
    for c in range(NCORES):
        xc_ = xs[c]
        x_pm = np.ascontiguousarray(
            xc_.reshape(J, P, 4).transpose(1, 0, 2)
        )  # [128, 256, 4]
        xg = np.ascontiguousarray(
            xc_[:, [0, 1]].reshape(M16, 16, 2).transpose(1, 0, 2)
        )  # [16, 2048, 2]
        xh = np.ascontiguousarray(
            xc_[:, [3, 2]].reshape(M16, 16, 2).transpose(1, 0, 2)
        )
        H16 = M16 // 2
        xq = np.concatenate(
            [xg[:, :H16], xg[:, H16:], xh[:, :H16], xh[:, H16:]], axis=0
        )  # [64, 1024, 2]
        in_maps.append(
            {
                "x_pm": x_pm,
                "xq": xq,
                "c0t": c0t,
                "c1f": c1f,
                "c3f": c3f,
                "c2t": c2t,
            }
        )
    return in_maps


def kernel(x, core0, core1, core2, core3):
    global _CACHED
    if _CACHED is None:
        _CACHED = _build_nc()
    nc = _CACHED
    in_maps = _prep_inputs(x, core0, core1, core2, core3)
    res = run_bass_kernel_spmd(nc, in_maps, core_ids=list(range(NCORES)))
    outs = []
    for c in range(NCORES):
        y_pm = res.results[c]["y_pm"]          # [128, 256]
        outs.append(np.ascontiguousarray(np.asarray(y_pm).T).reshape(-1))
    return np.concatenate(outs).astype(np.float32)

